# revision 3
# baseline (speedup 1.0000x reference)
"""Trainium2 Bass kernel for BaselineProtonet (retrieval_knn).

logits[q, c] = -||query_q - proto_c||_2
  proto_c = mean of 64 support embeddings of class c
  embeddings_stacked: [64 classes * (64 support + 64 query), 1024] f32

Sharding (8 cores): class-sharded. Core i owns classes 8i..8i+8: its 512
support rows (computes 8 prototypes locally) and its 512 query rows.
Prototypes are replicated via one AllGather (bf16, transposed layout),
then each core computes logits for its query shard with one PSUM-
accumulated matmul chain:

  dist^2[c, q] = ||q||^2 - 2 q.p + ||p||^2
    -2 q.p  : 8 accumulating matmuls, lhsT = -2*P^T chunk [128d, 64c]
              (bf16, built on device, AllGathered), rhs = Q^T chunk
              [128d, 512q] (bf16; Q^T is produced by the host-side shard
              step so everything on device stays feature-major - no
              transposes of Q needed)
    ||q||^2 : colsum matmul of (Q^T)^2 with ones stationary -> [1, 512],
              added via a K=1 fp32 matmul broadcasting over classes
    ||p||^2 : colsums of W^2/4 -> [1, 64], added via K=1 fp32 matmul
              broadcasting over queries
  logits = -sqrt(dist^2)  (ACT sqrt + DVE negate), output [64, 512] per
  core; host transposes/concats to [4096, 64].
"""

import numpy as np

C = 64          # classes
S = 64          # support per class (== queries per class)
D = 1024        # embedding dim
NCORES = 8
CL = C // NCORES            # 8 classes per core
SL = CL * S                 # 512 support rows per core
QL = CL * S                 # 512 query rows per core
DCH = D // 128              # 8 d-chunks
SCH = SL // 128             # 4 support row chunks

_CACHE = {}


def _emit(nc, tc, sup, qt, out):
    """Emit the per-core tile program.

    sup: [SL, D] f32 DRAM   (support rows of this core's 8 classes)
    qt:  [D, QL] f32 DRAM   (this core's queries, pre-transposed on host)
    out: [C, QL] f32 DRAM   (negated distances, class-major)
    """
    from concourse import masks, mybir

    f32 = mybir.dt.float32
    bf16 = mybir.dt.bfloat16
    AF = mybir.ActivationFunctionType

    with (
        tc.tile_pool(name="sb", bufs=1) as sb,
        tc.tile_pool(name="ps", bufs=1, space="PSUM") as ps,
        tc.tile_pool(name="dram", bufs=1, space="DRAM") as dram,
    ):
        # ---------------- input DMAs (sync ring: support first) --------
        s32 = sb.tile([128, SCH, D], f32)
        for j in range(SCH):
            nc.sync.dma_start(s32[:, j], sup[128 * j : 128 * (j + 1), :])
        qt32 = sb.tile([128, DCH, QL], f32)
        for k in range(DCH):
            nc.sync.dma_start(qt32[:, k], qt[128 * k : 128 * (k + 1), :])

        # ---------------- constants -----------------------------------
        oh = sb.tile([128, SCH, CL], bf16)  # per-s-chunk one-hot columns
        nc.gpsimd.memset(oh[:], 0.0)
        for j in range(SCH):
            for h in range(2):
                c = 2 * j + h
                nc.gpsimd.memset(
                    oh[64 * h : 64 * (h + 1), j : j + 1, c : c + 1], 1.0
                )
        ident = sb.tile([128, 128], f32)
        masks.make_identity(nc, ident[:])
        ones_col = sb.tile([128, 1], bf16)
        nc.vector.memset(ones_col[:], 1.0)
        ones_m = sb.tile([1, C], f32)
        nc.vector.memset(ones_m[:], 1.0)
        ones_q = sb.tile([1, QL], f32)
        nc.vector.memset(ones_q[:], 1.0)

        # ---------------- local prototypes ----------------------------
        # P_local[c, d] = sum_s sup[s, d] for s in class c  (8 classes)
        s16 = sb.tile([128, SCH, D], bf16)
        for j in range(SCH):
            nc.vector.tensor_copy(s16[:, j], s32[:, j])
        p_ps = ps.tile([CL, D], f32)  # [8, 1024] = 2 banks
        for j in range(SCH):
            for h in range(2):
                nc.tensor.matmul(
                    p_ps[:, 512 * h : 512 * (h + 1)],
                    oh[:, j],
                    s16[:, j, 512 * h : 512 * (h + 1)],
                    start=(j == 0),
                    stop=(j == SCH - 1),
                )
        psb = sb.tile([CL, D], f32)
        nc.scalar.mul(psb[:], p_ps[:], 1.0 / S)  # true prototypes (f32)

        # ---------------- transpose local protos -> -2*P^T bf16 -------
        pt_ps = ps.tile([128, CL * DCH], f32)  # chunk k at cols 8k..8k+8
        for k in range(DCH):
            nc.tensor.transpose(
                pt_ps[:, CL * k : CL * (k + 1)],
                psb[:, 128 * k : 128 * (k + 1)],
                ident[0:CL, 0:CL],
            )
        ptl = sb.tile([128, DCH * CL], bf16)
        nc.vector.tensor_scalar_mul(ptl[:], pt_ps[:], -2.0)

        # ---------------- AllGather of -2*P^T --------------------------
        cc_in = dram.tile([D, CL], bf16)
        cc_out = dram.tile([NCORES * D, CL], bf16)
        for k in range(DCH):
            nc.scalar.dma_start(
                cc_in[128 * k : 128 * (k + 1), :], ptl[:, CL * k : CL * (k + 1)]
            )
        nc.gpsimd.collective_compute(
            "AllGather",
            mybir.AluOpType.bypass,
            replica_groups=[list(range(NCORES))],
            ins=[cc_in.opt()],
            outs=[cc_out.opt()],
        )
        # W[d_local, k, r, c] = -2 * proto^T, classes in global order r*8+c
        W = sb.tile([128, DCH, NCORES, CL], bf16)
        for r in range(NCORES):
            src = cc_out[r * D : (r + 1) * D, :].rearrange(
                "(k d) c -> d k c", d=128
            )
            nc.scalar.dma_start(W[:, :, r, :], src)

        # ---------------- query side -----------------------------------
        q16 = sb.tile([128, DCH, QL], bf16)
        qsq = sb.tile([128, DCH, QL], bf16)
        for k in range(DCH):
            nc.vector.tensor_copy(q16[:, k], qt32[:, k])
        for k in range(DCH):
            if k % 2 == 0:
                nc.vector.tensor_mul(qsq[:, k], q16[:, k], q16[:, k])
            else:
                nc.scalar.square(qsq[:, k], q16[:, k])
        qn_ps = ps.tile([1, QL], f32)
        for k in range(DCH):
            nc.tensor.matmul(
                qn_ps[:], ones_col[:], qsq[:, k], start=(k == 0), stop=(k == DCH - 1)
            )
        qn_row = sb.tile([1, QL], f32)
        nc.scalar.copy(qn_row[:], qn_ps[:])

        # ---------------- ||p||^2 from W (post-AG, bf16-consistent) ----
        wsq = sb.tile([128, DCH, NCORES, CL], bf16)
        nc.vector.tensor_mul(wsq[:], W[:], W[:])
        pn_ps = ps.tile([1, DCH * C], f32)  # [1, 512] per-(k, class) colsums
        nc.tensor.matmul(
            pn_ps[:],
            ones_col[:],
            wsq[:].rearrange("p k r c -> p (k r c)"),
            start=True,
            stop=True,
        )
        pn_row = sb.tile([1, C], f32)
        nc.vector.tensor_reduce(
            pn_row[:],
            pn_ps[:].rearrange("p (k c) -> p c k", c=C),
            axis=mybir.AxisListType.X,
            op=mybir.AluOpType.add,
        )
        nc.vector.tensor_scalar_mul(pn_row[:], pn_row[:], 0.25)  # W=-2p -> /4

        # ---------------- Gram + norm augmentation ---------------------
        s_ps = ps.tile([C, QL], f32)
        for k in range(DCH):
            nc.tensor.matmul(
                s_ps[:], W[:, k], q16[:, k], start=(k == 0), stop=False
            )
        nc.tensor.matmul(s_ps[:], pn_row[:], ones_q[:], start=False, stop=False)
        nc.tensor.matmul(s_ps[:], ones_m[:], qn_row[:], start=False, stop=True)

        # ---------------- sqrt, negate, store ---------------------------
        lt = sb.tile([C, QL], f32)
        nc.scalar.activation(lt[:], s_ps[:], AF.Sqrt)
        nc.vector.tensor_scalar_mul(lt[:], lt[:], -1.0)
        nc.scalar.dma_start(out[:, :], lt[:])


def _build():
    if "nc" in _CACHE:
        return _CACHE["nc"]
    from concourse import bacc, mybir, tile

    f32 = mybir.dt.float32
    nc = bacc.Bacc(
        "TRN2",
        target_bir_lowering=False,
        debug=False,
        enable_asserts=False,
        num_devices=NCORES,
    )
    sup = nc.dram_tensor("sup", [SL, D], f32, kind="ExternalInput").ap()
    qt = nc.dram_tensor("qt", [D, QL], f32, kind="ExternalInput").ap()
    out = nc.dram_tensor("out", [C, QL], f32, kind="ExternalOutput").ap()
    with tile.TileContext(nc) as tc:
        _emit(nc, tc, sup, qt, out)
    nc.compile()
    _CACHE["nc"] = nc
    return nc


def _shard(embeddings):
    emb = np.ascontiguousarray(np.asarray(embeddings, dtype=np.float32)).reshape(
        C, 2 * S, D
    )
    in_maps = []
    for i in range(NCORES):
        cls = emb[CL * i : CL * (i + 1)]  # [8, 128, 1024]
        sup_i = np.ascontiguousarray(cls[:, :S, :].reshape(SL, D))
        qt_i = np.ascontiguousarray(cls[:, S:, :].reshape(QL, D).T)
        in_maps.append({"sup": sup_i, "qt": qt_i})
    return in_maps


def kernel(embeddings_stacked, n_classes, n_support, **_unused):
    assert int(n_classes) == C and int(n_support) == S
    emb = np.asarray(embeddings_stacked)
    assert emb.shape == (C * 2 * S, D), emb.shape

    from concourse import bass_utils

    nc = _build()
    in_maps = _shard(emb)
    res = bass_utils.run_bass_kernel_spmd(nc, in_maps, core_ids=list(range(NCORES)))
    logits = np.empty((C * S, C), dtype=np.float32)
    for i in range(NCORES):
        logits[QL * i : QL * (i + 1), :] = res.results[i]["out"].T
    return logits


if __name__ == "__main__":
    rng = np.random.default_rng(0)
    emb = rng.standard_normal((C * 2 * S, D), dtype=np.float32)
    got = kernel(emb, C, S)
    print("kernel output", got.shape, got.dtype)


# revision 5
# speedup vs baseline: 1.9650x; 1.9650x over previous
"""Trainium2 Bass kernel for BaselineProtonet (retrieval_knn).

logits[q, c] = -||query_q - proto_c||_2
  proto_c = mean of 64 support embeddings of class c
  embeddings_stacked: [64 classes * (64 support + 64 query), 1024] f32

Sharding (8 cores): query-sharded, support-replicated. Core i owns query
rows 512i..512(i+1); every core receives the full support set (bf16 on
the wire) and computes all 64 prototypes locally on the TensorEngine, so
no cross-core collective is needed (a ncfw collective costs ~50us of
control latency in this runtime, far more than the extra ~7MB of DMA).

Per core:
  protos   : 64 one-hot matmuls accumulate class sums -> PSUM [64,1024],
             scaled 1/64 on evacuation (ACT) -> bf16 prototypes
  P^T      : 8 PE transposes -> W = -2*P^T (bf16, DVE evac with scale)
  ||p||^2  : DVE tensor_tensor_reduce on prototypes -> [64,1] f32,
             added per-partition (class) via the ACT sqrt bias
  ||q||^2  : Q^T arrives bf16 pre-transposed from host; squares (DVE) +
             ones-stationary colsum matmuls -> [1,512] f32, added via a
             K=1 fp32 matmul broadcast over classes
  Gram     : 8 accumulating matmuls lhsT=W chunk, rhs=Q^T chunk
  logits   : -sqrt(dist^2) via ACT sqrt(+bias) and DVE negate,
             output [64, 512] (class-major); host transposes/concats.
"""

import numpy as np

C = 64          # classes
S = 64          # support per class (== queries per class)
D = 1024        # embedding dim
NCORES = 8
CL = C // NCORES            # 8 classes per core's query shard
QL = CL * S                 # 512 query rows per core
DCH = D // 128              # 8 d-chunks
SCH = (C * S) // 128        # 32 support row chunks (full support)

_CACHE = {}


def _emit(nc, tc, sup, qt, oh_in, out):
    """Emit the per-core tile program.

    sup:   [C*S, D] bf16 DRAM   (full support set)
    qt:    [D, QL] bf16 DRAM    (this core's queries, pre-transposed)
    oh_in: [128, SCH*C] bf16 DRAM (one-hot class masks per row chunk)
    out:   [C, QL] f32 DRAM     (negated distances, class-major)
    """
    from concourse import masks, mybir

    f32 = mybir.dt.float32
    bf16 = mybir.dt.bfloat16
    AF = mybir.ActivationFunctionType

    with (
        tc.tile_pool(name="sb", bufs=1) as sb,
        tc.tile_pool(name="ps", bufs=1, space="PSUM") as ps,
    ):
        # ---------------- input DMAs ------------------------------------
        # sync ring: support stream (4 batches of 8 chunks)
        s16 = sb.tile([128, SCH, D], bf16)
        sup_r = sup.rearrange("(c p) d -> p c d", p=128)
        for b in range(4):
            nc.sync.dma_start(s16[:, 8 * b : 8 * (b + 1)], sup_r[:, 8 * b : 8 * (b + 1)])
        # scalar ring: one-hot masks + queries (small, parallel)
        oh = sb.tile([128, SCH, C], bf16)
        nc.scalar.dma_start(oh[:], oh_in[:, :].rearrange("p (c k) -> p c k", c=SCH))
        q16 = sb.tile([128, DCH, QL], bf16)
        nc.scalar.dma_start(q16[:], qt[:, :].rearrange("(c p) q -> p c q", p=128))

        # ---------------- constants -------------------------------------
        ident = sb.tile([128, 128], bf16)
        masks.make_identity(nc, ident[:])
        ones_col = sb.tile([128, 1], bf16)
        nc.gpsimd.memset(ones_col[:], 1.0)
        ones_m = sb.tile([1, C], f32)
        nc.gpsimd.memset(ones_m[:], 1.0)

        # ---------------- prototypes (all 64 classes) -------------------
        p_ps = ps.tile([C, D], f32)  # [64, 1024] = 2 banks
        for j in range(SCH):
            for h in range(2):
                nc.tensor.matmul(
                    p_ps[:, 512 * h : 512 * (h + 1)],
                    oh[:, j],
                    s16[:, j, 512 * h : 512 * (h + 1)],
                    start=(j == 0),
                    stop=(j == SCH - 1),
                )
        psb = sb.tile([C, D], bf16)
        nc.scalar.mul(psb[:], p_ps[:], 1.0 / S)  # prototypes, bf16

        # ||p||^2 in f32 (consistent with bf16 protos used in the Gram)
        pn_dump = sb.tile([C, D], bf16)
        pn_col = sb.tile([C, 1], f32)
        nc.vector.tensor_mul(pn_dump[:], psb[:], psb[:])
        nc.vector.tensor_reduce(
            pn_col[:], pn_dump[:], axis=mybir.AxisListType.X, op=mybir.AluOpType.add
        )

        # ---------------- W = -2 * P^T (bf16) ---------------------------
        pt_ps = ps.tile([128, DCH * C], bf16)  # chunk k at cols 64k..64k+64
        for k in range(DCH):
            nc.tensor.transpose(
                pt_ps[:, C * k : C * (k + 1)],
                psb[:, 128 * k : 128 * (k + 1)],
                ident[0:C, 0:C],
            )
        W = sb.tile([128, DCH, C], bf16)
        nc.vector.tensor_scalar_mul(W[:], pt_ps[:], -2.0)

        # ---------------- ||q||^2 ---------------------------------------
        qsq = sb.tile([128, DCH, QL], bf16)
        nc.vector.tensor_mul(qsq[:], q16[:], q16[:])
        qn_ps = ps.tile([1, QL], f32)
        for k in range(DCH):
            nc.tensor.matmul(
                qn_ps[:], ones_col[:], qsq[:, k], start=(k == 0), stop=(k == DCH - 1)
            )
        qn_row = sb.tile([1, QL], f32)
        nc.scalar.copy(qn_row[:], qn_ps[:])

        # ---------------- Gram + ||q||^2 augmentation -------------------
        s_ps = ps.tile([C, QL], f32)
        for k in range(DCH):
            nc.tensor.matmul(s_ps[:], W[:, k], q16[:, k], start=(k == 0), stop=False)
        nc.tensor.matmul(s_ps[:], ones_m[:], qn_row[:], start=False, stop=True)

        # ---------------- sqrt(+||p||^2), negate, store ------------------
        lt = sb.tile([C, QL], f32)
        nc.scalar.activation(lt[:], s_ps[:], AF.Sqrt, bias=pn_col[:, 0:1])
        nc.vector.tensor_scalar_mul(lt[:], lt[:], -1.0)
        nc.scalar.dma_start(out[:, :], lt[:])


def _build():
    if "nc" in _CACHE:
        return _CACHE["nc"]
    from concourse import bacc, mybir, tile

    f32 = mybir.dt.float32
    bf16 = mybir.dt.bfloat16
    nc = bacc.Bacc(
        "TRN2",
        target_bir_lowering=False,
        debug=False,
        enable_asserts=False,
        num_devices=NCORES,
    )
    sup = nc.dram_tensor("sup", [C * S, D], bf16, kind="ExternalInput").ap()
    qt = nc.dram_tensor("qt", [D, QL], bf16, kind="ExternalInput").ap()
    oh_in = nc.dram_tensor("oh", [128, SCH * C], bf16, kind="ExternalInput").ap()
    out = nc.dram_tensor("out", [C, QL], f32, kind="ExternalOutput").ap()
    with tile.TileContext(nc) as tc:
        _emit(nc, tc, sup, qt, oh_in, out)
    nc.compile()
    _CACHE["nc"] = nc
    return nc


def _onehot():
    import ml_dtypes

    # oh[p, j, c] = 1 iff class c == 2j + p//64 (row chunk j covers
    # support rows 128j..128j+128 = classes 2j, 2j+1)
    p = np.arange(128)[:, None, None]
    j = np.arange(SCH)[None, :, None]
    c = np.arange(C)[None, None, :]
    oh = (c == 2 * j + p // 64).astype(ml_dtypes.bfloat16)
    return np.ascontiguousarray(oh.reshape(128, SCH * C))


def _shard(embeddings):
    import ml_dtypes

    bf16 = ml_dtypes.bfloat16
    emb = np.asarray(embeddings, dtype=np.float32).reshape(C, 2 * S, D)
    sup = np.ascontiguousarray(emb[:, :S, :].reshape(C * S, D).astype(bf16))
    oh = _onehot()
    in_maps = []
    for i in range(NCORES):
        q = emb[CL * i : CL * (i + 1), S:, :].reshape(QL, D)
        qt_i = np.ascontiguousarray(q.T.astype(bf16))
        in_maps.append({"sup": sup, "qt": qt_i, "oh": oh})
    return in_maps


def kernel(embeddings_stacked, n_classes, n_support, **_unused):
    assert int(n_classes) == C and int(n_support) == S
    emb = np.asarray(embeddings_stacked)
    assert emb.shape == (C * 2 * S, D), emb.shape

    from concourse import bass_utils

    nc = _build()
    in_maps = _shard(emb)
    res = bass_utils.run_bass_kernel_spmd(nc, in_maps, core_ids=list(range(NCORES)))
    logits = np.empty((C * S, C), dtype=np.float32)
    for i in range(NCORES):
        logits[QL * i : QL * (i + 1), :] = res.results[i]["out"].T
    return logits


if __name__ == "__main__":
    rng = np.random.default_rng(0)
    emb = rng.standard_normal((C * 2 * S, D), dtype=np.float32)
    got = kernel(emb, C, S)
    print("kernel output", got.shape, got.dtype)


# revision 6
# speedup vs baseline: 2.5901x; 1.3181x over previous
"""Trainium2 Bass kernel for BaselineProtonet (retrieval_knn).

logits[q, c] = -||query_q - proto_c||_2
  proto_c = mean of 64 support embeddings of class c
  embeddings_stacked: [64 classes * (64 support + 64 query), 1024] f32

Sharding (8 cores): query-sharded, support-replicated. Core i owns query
rows 512i..512(i+1); every core receives the full support set (fp8 on
the wire) and computes all 64 prototypes locally on the TensorEngine, so
no cross-core collective is needed (a ncfw collective costs ~50us of
control latency in this runtime, far more than the extra DMA).

Host-side shard prep (layout/encoding only, no arithmetic): support is
pre-swizzled to the exact SBUF layout (contiguous per-partition runs so
HWDGE descriptor generation is cheap) and encoded fp8e4m3; queries are
transposed to feature-major (d on partitions) and encoded bf16.

Per core:
  protos   : 64 one-hot matmuls (fp8) accumulate class sums -> PSUM
             [64,1024] f32, scaled 1/64 on evacuation -> bf16 prototypes
  P^T      : 8 PE transposes -> W = -2*P^T (bf16)
  ||p||^2  : DVE square + reduce on prototypes -> [64,1] f32, added
             per-partition (class) via the ACT sqrt bias
  ||q||^2  : DVE squares + ones-stationary colsum matmuls -> [1,512]
             f32, added via a K=1 fp32 matmul broadcast over classes
  Gram     : 8 accumulating matmuls lhsT=W chunk, rhs=Q^T chunk (bf16)
  logits   : -sqrt(dist^2) via ACT sqrt(+bias) and DVE negate,
             output [64, 512] (class-major); host transposes/concats.
PE is pre-warmed with dummy matmuls during the DMA wait (HAM clock gate)
and the sqrt ACT table is preloaded by a dummy activation.
"""

import numpy as np

C = 64          # classes
S = 64          # support per class (== queries per class)
D = 1024        # embedding dim
NCORES = 8
CL = C // NCORES            # 8 classes per core's query shard
QL = CL * S                 # 512 query rows per core
DCH = D // 128              # 8 d-chunks
SCH = (C * S) // 128        # 32 support row chunks (full support)

_CACHE = {}


def _emit(nc, tc, sup, qt, oh_in, out):
    """Emit the per-core tile program.

    sup:   [128, SCH*D] fp8 DRAM  (full support, swizzled: row p holds
                                   sup[j*128+p, :] for j = 0..31)
    qt:    [128, DCH*QL] bf16 DRAM (queries, swizzled feature-major)
    oh_in: [128, SCH*C] fp8 DRAM  (one-hot class masks per row chunk)
    out:   [C, QL] f32 DRAM       (negated distances, class-major)
    """
    from concourse import masks, mybir

    f32 = mybir.dt.float32
    bf16 = mybir.dt.bfloat16
    fp8 = mybir.dt.float8e4
    AF = mybir.ActivationFunctionType

    with (
        tc.tile_pool(name="sb", bufs=1) as sb,
        tc.tile_pool(name="ps", bufs=1, space="PSUM") as ps,
    ):
        # ---------------- input DMAs ------------------------------------
        # sync ring: support stream (4 slices of 8 chunks, contiguous rows)
        s8 = sb.tile([128, SCH, D], fp8)
        for b in range(4):
            nc.sync.dma_start(
                s8[:, 8 * b : 8 * (b + 1)],
                sup[:, 8 * b * D : 8 * (b + 1) * D].rearrange(
                    "p (c d) -> p c d", c=8
                ),
            )
        # scalar ring: one-hot masks + queries (parallel with support)
        oh = sb.tile([128, SCH, C], fp8)
        nc.scalar.dma_start(oh[:], oh_in[:, :].rearrange("p (c k) -> p c k", c=SCH))
        q16 = sb.tile([128, DCH, QL], bf16)
        nc.scalar.dma_start(q16[:], qt[:, :].rearrange("p (k q) -> p k q", k=DCH))

        # ---------------- constants -------------------------------------
        ident = sb.tile([128, 128], bf16)
        masks.make_identity(nc, ident[:])
        ones_col = sb.tile([128, 1], bf16)
        nc.gpsimd.memset(ones_col[:], 1.0)
        ones_m = sb.tile([1, C], f32)
        nc.gpsimd.memset(ones_m[:], 1.0)

        # preload the sqrt ACT table set off the critical path
        warm_sq = sb.tile([1, 1], f32)
        nc.gpsimd.memset(warm_sq[:], 1.0)
        nc.scalar.activation(warm_sq[:], warm_sq[:], AF.Sqrt)

        # warm the PE clock (HAM gate needs ~3.5us of busy) while the
        # support DMA streams in
        wm_ps = ps.tile([128, 128], f32)
        for _ in range(28):
            nc.tensor.matmul(wm_ps[:], ident[:], ident[:], start=True, stop=True)

        # ---------------- prototypes (all 64 classes) -------------------
        p_ps = ps.tile([C, D], f32)  # [64, 1024] = 2 banks
        for j in range(SCH):
            for h in range(2):
                nc.tensor.matmul(
                    p_ps[:, 512 * h : 512 * (h + 1)],
                    oh[:, j],
                    s8[:, j, 512 * h : 512 * (h + 1)],
                    start=(j == 0),
                    stop=(j == SCH - 1),
                )
        psb = sb.tile([C, D], bf16)
        nc.scalar.mul(psb[:], p_ps[:], 1.0 / S)  # prototypes, bf16

        # ||p||^2 in f32 (consistent with bf16 protos used in the Gram)
        pn_dump = sb.tile([C, D], bf16)
        pn_col = sb.tile([C, 1], f32)
        nc.vector.tensor_mul(pn_dump[:], psb[:], psb[:])
        nc.vector.tensor_reduce(
            pn_col[:], pn_dump[:], axis=mybir.AxisListType.X, op=mybir.AluOpType.add
        )

        # ---------------- W = -2 * P^T (bf16) ---------------------------
        pt_ps = ps.tile([128, DCH * C], bf16)  # chunk k at cols 64k..64k+64
        for k in range(DCH):
            nc.tensor.transpose(
                pt_ps[:, C * k : C * (k + 1)],
                psb[:, 128 * k : 128 * (k + 1)],
                ident[0:C, 0:C],
            )
        W = sb.tile([128, DCH, C], bf16)
        nc.vector.tensor_scalar_mul(W[:], pt_ps[:], -2.0)

        # ---------------- ||q||^2 ---------------------------------------
        qsq = sb.tile([128, DCH, QL], bf16)
        nc.vector.tensor_mul(qsq[:], q16[:], q16[:])
        qn_ps = ps.tile([1, QL], f32)
        for k in range(DCH):
            nc.tensor.matmul(
                qn_ps[:], ones_col[:], qsq[:, k], start=(k == 0), stop=(k == DCH - 1)
            )
        qn_row = sb.tile([1, QL], f32)
        nc.scalar.copy(qn_row[:], qn_ps[:])

        # ---------------- Gram + ||q||^2 augmentation -------------------
        s_ps = ps.tile([C, QL], f32)
        for k in range(DCH):
            nc.tensor.matmul(s_ps[:], W[:, k], q16[:, k], start=(k == 0), stop=False)
        nc.tensor.matmul(s_ps[:], ones_m[:], qn_row[:], start=False, stop=True)

        # ---------------- sqrt(+||p||^2), negate, store ------------------
        lt = sb.tile([C, QL], f32)
        nc.scalar.activation(lt[:], s_ps[:], AF.Sqrt, bias=pn_col[:, 0:1])
        nc.vector.tensor_scalar_mul(lt[:], lt[:], -1.0)
        nc.scalar.dma_start(out[:, :], lt[:])


def _build():
    if "nc" in _CACHE:
        return _CACHE["nc"]
    from concourse import bacc, mybir, tile

    f32 = mybir.dt.float32
    bf16 = mybir.dt.bfloat16
    fp8 = mybir.dt.float8e4
    nc = bacc.Bacc(
        "TRN2",
        target_bir_lowering=False,
        debug=False,
        enable_asserts=False,
        num_devices=NCORES,
    )
    sup = nc.dram_tensor("sup", [128, SCH * D], fp8, kind="ExternalInput").ap()
    qt = nc.dram_tensor("qt", [128, DCH * QL], bf16, kind="ExternalInput").ap()
    oh_in = nc.dram_tensor("oh", [128, SCH * C], fp8, kind="ExternalInput").ap()
    out = nc.dram_tensor("out", [C, QL], f32, kind="ExternalOutput").ap()
    with tile.TileContext(nc) as tc:
        _emit(nc, tc, sup, qt, oh_in, out)
    nc.compile()
    _CACHE["nc"] = nc
    return nc


def _onehot():
    import ml_dtypes

    # oh[p, j, c] = 1 iff class c == 2j + p//64 (row chunk j covers
    # support rows 128j..128j+128 = classes 2j, 2j+1)
    p = np.arange(128)[:, None, None]
    j = np.arange(SCH)[None, :, None]
    c = np.arange(C)[None, None, :]
    oh = (c == 2 * j + p // 64).astype(ml_dtypes.float8_e4m3)
    return np.ascontiguousarray(oh.reshape(128, SCH * C))


def _shard(embeddings):
    import ml_dtypes

    emb = np.asarray(embeddings, dtype=np.float32).reshape(C, 2 * S, D)
    # support: [C*S, D] -> swizzled [128, SCH, D] (row p of chunk j =
    # support row j*128+p), fp8 on the wire
    sup = emb[:, :S, :].reshape(SCH, 128, D).transpose(1, 0, 2)
    sup = np.ascontiguousarray(
        sup.astype(ml_dtypes.float8_e4m3).reshape(128, SCH * D)
    )
    oh = _onehot()
    in_maps = []
    for i in range(NCORES):
        q = emb[CL * i : CL * (i + 1), S:, :].reshape(QL, D)
        # Q^T [D, QL] -> swizzled [128, DCH, QL] bf16
        qt_i = q.T.reshape(DCH, 128, QL).transpose(1, 0, 2)
        qt_i = np.ascontiguousarray(
            qt_i.astype(ml_dtypes.bfloat16).reshape(128, DCH * QL)
        )
        in_maps.append({"sup": sup, "qt": qt_i, "oh": oh})
    return in_maps


def kernel(embeddings_stacked, n_classes, n_support, **_unused):
    assert int(n_classes) == C and int(n_support) == S
    emb = np.asarray(embeddings_stacked)
    assert emb.shape == (C * 2 * S, D), emb.shape

    from concourse import bass_utils

    nc = _build()
    in_maps = _shard(emb)
    res = bass_utils.run_bass_kernel_spmd(nc, in_maps, core_ids=list(range(NCORES)))
    logits = np.empty((C * S, C), dtype=np.float32)
    for i in range(NCORES):
        logits[QL * i : QL * (i + 1), :] = res.results[i]["out"].T
    return logits


if __name__ == "__main__":
    rng = np.random.default_rng(0)
    emb = rng.standard_normal((C * 2 * S, D), dtype=np.float32)
    got = kernel(emb, C, S)
    print("kernel output", got.shape, got.dtype)
